# revision 14
# baseline (speedup 1.0000x reference)
"""Trainium2 Bass kernel for nn_GPU_Actor (gnn_message_passing).

Math (H=1 collapses the whole network to per-row scalars):
  Edot[b,i] = expert_node[b,i,:] . W_expert[0,:]
  Gdot[b,i] = gpu_nodes[b,i,:]  . W_gpu[0,:]
  A[b,i]  = sum_j affinity[b,i,j]   (same for bandwidth/traffic)
  h[b,i] = relu( c_pre_e*Edot + c_pre_g*Gdot + c_k0_e*Se + c_k0_g*Sg
                 + k_a*A + k_b*Bs + k_t*Ts )
  out[b,i,g] = mask[b,i,g] ? 0 : exp(h[b,i]*W2[g]) / Z[b,i]
  Z[b,i] = sum_g (1-mask) * exp(h[b,i]*W2[g])

Memory-bound; byte-count is everything:
  - affinity/bandwidth/traffic only enter via row sums -> shipped as
    fp8(e3m4), host-transposed to [j, i] layout so TensorE does the row
    sums as ones-vector matmuls (PSUM accumulation over j-chunks). The
    k_a/k_b/k_t coefficients are folded into per-tensor power-of-two fp8
    stationaries (exact) with the residual ratio folded into the fp8 data,
    so one PSUM bank accumulates k_a*A + k_b*Bs + k_t*Ts directly. The 4
    i-chunk matmuls go to distinct 32-col strips (tile_position) so they
    overlap on the array.
  - mask ships uint8, output is fp16 (host-upcast). ~50MB DMA per core.
  - All big tensors are host-permuted so every DMA descriptor is a >=4KB
    contiguous run, and loads/masks/stores ride one HWDGE ring in
    execution order (emission order = drain order, no round-robin
    dilution of the critical input stream).

Sharding: data-parallel over batch B=16 across 8 cores (2 batches/core).
"""
import sys

sys.path.insert(0, '/opt/trn_rl_repo')

import ml_dtypes
import numpy as np

import concourse.bacc as bacc
import concourse.mybir as mybir
from concourse.bass_isa import ReduceOp
from concourse.bass_utils import run_bass_kernel_spmd
from concourse.tile import TileContext

B, N, DE, DG = 16, 2048, 16, 8
NCORES = 8
BB = B // NCORES          # batches per core
P = 128                   # partitions
TILES = N // P            # 16 row-tiles per batch
JG = 4                    # j-chunks per input DMA (512 rows)
NJG = TILES // JG         # input DMAs per (batch-tensor, i-half)
NH = 2                    # i-halves: row sums finish per half
N2 = N // NH              # 1024
IC = 4                    # 512-col chunks -> 4 PE col strips
ICH = IC // NH            # col strips per half
FC = N // IC              # 512
MG = 2                    # row-tiles per output work group
NMG = TILES // MG         # 8 groups per batch
TPH = TILES // NH         # row-tiles per half

f32 = mybir.dt.float32
f16 = mybir.dt.float16
f8d = mybir.dt.float8e3   # e3m4: data dtype (4 mantissa bits)
f8s = mybir.dt.float8e5   # e5m2: stationary dtype (pow2 exact)
u8 = mybir.dt.uint8
AX = mybir.AxisListType
OP = mybir.AluOpType
AF = mybir.ActivationFunctionType

NP_F8D = ml_dtypes.float8_e3m4
NP_F8S = ml_dtypes.float8_e5m2
F8_CLIP = 15.0            # e3m4 max normal is 15.5
STAT_EMIN, STAT_EMAX = -14, 15


def _build_nc(consts):
    c_pre_e = float(consts["c_pre_e"])
    c_pre_g = float(consts["c_pre_g"])
    c_k0_e = float(consts["c_k0_e"])
    c_k0_g = float(consts["c_k0_g"])

    nc = bacc.Bacc("TRN2", target_bir_lowering=False, debug=False,
                   num_devices=NCORES)

    # inputs, host-permuted:
    #   big fp8 [BB, NH, NJG, P, JG, N2]: row (jg*JG*P + u*P + p), col
    #     (h*N2 + n) of the transposed [j, i] tensor at [b, h, jg, p, u, n]
    #     -- i-halves stored separately so the low half's row sums (and
    #     thus h[0:1024]) complete after only half the input bytes.
    #   mask [BB, NMG, P, MG, N]: row (g*MG*P + u*P + p) at [b, g, p, u, :]
    #   out  [BB, NMG, P, MG, N] fp16, same permutation (host undoes it)
    a8 = nc.dram_tensor("aff8", [BB, NH, NJG, P, JG, N2], f8d,
                        kind="ExternalInput")
    b8 = nc.dram_tensor("bwd8", [BB, NH, NJG, P, JG, N2], f8d,
                        kind="ExternalInput")
    t8 = nc.dram_tensor("trf8", [BB, NH, NJG, P, JG, N2], f8d,
                        kind="ExternalInput")
    msk = nc.dram_tensor("mask", [BB, NMG, P, MG, N], u8,
                         kind="ExternalInput")
    xe = nc.dram_tensor("xe", [BB, P, TILES, DE], f32, kind="ExternalInput")
    xg = nc.dram_tensor("xg", [BB, P, TILES, DG], f32, kind="ExternalInput")
    w2b = nc.dram_tensor("w2b", [P, N], f16, kind="ExternalInput")
    ueb = nc.dram_tensor("ueb", [P, TILES, DE], f32, kind="ExternalInput")
    ugb = nc.dram_tensor("ugb", [P, TILES, DG], f32, kind="ExternalInput")
    st8 = nc.dram_tensor("stat8", [P, 4], f8s, kind="ExternalInput")
    out_d = nc.dram_tensor("out", [BB, NMG, P, MG, N], f16,
                           kind="ExternalOutput")

    with TileContext(nc) as tc:
        with tc.tile_pool(name="const", bufs=1) as cpool, \
             tc.tile_pool(name="stream", bufs=3) as spool, \
             tc.tile_pool(name="mpool", bufs=6) as mpool, \
             tc.tile_pool(name="epool", bufs=4) as epool, \
             tc.tile_pool(name="opool", bufs=3) as opool, \
             tc.tile_pool(name="small", bufs=6) as smpool, \
             tc.tile_pool(name="psA", bufs=1, space="PSUM") as papool, \
             tc.tile_pool(name="psT", bufs=2, space="PSUM") as ptpool:

            w2b_sb = cpool.tile([P, N], f16, tag="w2b")
            nc.scalar.dma_start(w2b_sb[:], w2b[:])
            st_sb = cpool.tile([P, 4], f8s, tag="stat8")
            nc.scalar.dma_start(st_sb[:], st8[:])
            ones_sb = cpool.tile([P, 1], f32, tag="ones")
            nc.vector.memset(ones_sb[:], 1.0)
            ue_sb = cpool.tile([P, TILES, DE], f32, tag="ueb")
            nc.scalar.dma_start(ue_sb[:], ueb[:])
            ug_sb = cpool.tile([P, TILES, DG], f32, tag="ugb")
            nc.scalar.dma_start(ug_sb[:], ugb[:])

            # warm the ACT exp table before it's on the critical path
            warm = smpool.tile([P, 1], f32, tag="warm")
            nc.scalar.activation(out=warm[:], in_=ue_sb[:, 0, 0:1],
                                 func=AF.Exp, bias=0.0, scale=0.0)

            # ---- stage 1: per-batch row scalars from tiny xe/xg ----
            pre = []
            for b in range(BB):
                xe_sb = cpool.tile([P, TILES, DE], f32, tag=f"xe{b}")
                nc.scalar.dma_start(xe_sb[:], xe[b])
                xg_sb = cpool.tile([P, TILES, DG], f32, tag=f"xg{b}")
                nc.scalar.dma_start(xg_sb[:], xg[b])

                prod_e = smpool.tile([P, TILES, DE], f32, tag="prod_e")
                nc.vector.tensor_mul(out=prod_e[:], in0=xe_sb[:], in1=ue_sb[:])
                edot = cpool.tile([P, TILES], f32, tag=f"edot{b}")
                nc.vector.tensor_reduce(out=edot[:], in_=prod_e[:],
                                        axis=AX.X, op=OP.add)
                prod_g = smpool.tile([P, TILES, DG], f32, tag="prod_g")
                nc.vector.tensor_mul(out=prod_g[:], in0=xg_sb[:], in1=ug_sb[:])
                gdot = cpool.tile([P, TILES], f32, tag=f"gdot{b}")
                nc.vector.tensor_reduce(out=gdot[:], in_=prod_g[:],
                                        axis=AX.X, op=OP.add)

                sep = smpool.tile([P, 1], f32, tag="sep")
                nc.vector.tensor_reduce(out=sep[:], in_=edot[:],
                                        axis=AX.X, op=OP.add)
                sgp = smpool.tile([P, 1], f32, tag="sgp")
                nc.vector.tensor_reduce(out=sgp[:], in_=gdot[:],
                                        axis=AX.X, op=OP.add)
                sea = smpool.tile([P, 1], f32, tag="sea")
                nc.gpsimd.partition_all_reduce(sea[:], sep[:], channels=P,
                                               reduce_op=ReduceOp.add)
                sga = smpool.tile([P, 1], f32, tag="sga")
                nc.gpsimd.partition_all_reduce(sga[:], sgp[:], channels=P,
                                               reduce_op=ReduceOp.add)

                k0 = smpool.tile([P, 1], f32, tag="k0")
                nc.vector.tensor_scalar(out=k0[:], in0=sea[:],
                                        scalar1=c_k0_e, scalar2=None,
                                        op0=OP.mult)
                k0b = cpool.tile([P, 1], f32, tag=f"k0b{b}")
                nc.vector.tensor_scalar(out=k0b[:], in0=sga[:],
                                        scalar1=c_k0_g, scalar2=k0[:, 0:1],
                                        op0=OP.mult, op1=OP.add)
                pre_b = cpool.tile([P, TILES], f32, tag=f"pre{b}")
                nc.vector.tensor_scalar(out=pre_b[:], in0=edot[:],
                                        scalar1=c_pre_e, scalar2=k0b[:, 0:1],
                                        op0=OP.mult, op1=OP.add)
                nc.vector.scalar_tensor_tensor(out=pre_b[:], in0=gdot[:],
                                               scalar=c_pre_g, in1=pre_b[:],
                                               op0=OP.mult, op1=OP.add)
                pre.append(pre_b)

            # ---- phase A: TensorE row sums. One input DMA = [P, JG, N2]
            # fp8 (4KB contiguous per partition); per (x, jg, u) the two
            # strip matmuls of this half go to distinct 32-col strips of
            # the array and accumulate into psA[32*ic, :]. ----
            SRCS = (a8, b8, t8)

            def emit_a_load(b, hf, x, jg, psA):
                d_t = spool.tile([P, JG, N2], f8d, tag=f"in{x}")
                nc.sync.dma_start(d_t[:], SRCS[x][b, hf, jg])
                for u in range(JG):
                    for icl in range(ICH):
                        ic = hf * ICH + icl
                        nc.tensor.matmul(
                            psA[32 * ic:32 * ic + 1, :],
                            lhsT=st_sb[:, x:x + 1],
                            rhs=d_t[:, u, icl * FC:(icl + 1) * FC],
                            start=(x == 0 and jg == 0 and u == 0),
                            stop=(x == 2 and jg == NJG - 1 and u == JG - 1),
                            tile_position=(0, 32 * ic))

            hbs = {}

            def emit_plumb(b, hf, psA):
                # PSUM strips of this half -> SBUF (partition-aligned ACT
                # copies), then tiny PE transposes [1,128] -> [128,1]
                # spread the per-row sums across partitions; add pre, relu.
                rs4 = smpool.tile([P, FC], f32, tag=f"rs4_{hf}")
                for icl in range(ICH):
                    ic = hf * ICH + icl
                    nc.scalar.copy(rs4[32 * ic:32 * ic + 1, :],
                                   psA[32 * ic:32 * ic + 1, :])
                psT = ptpool.tile([P, TPH], f32, tag="psT")
                for tl in range(TPH):
                    t = hf * TPH + tl
                    ic = t // IC
                    off = (t % IC) * P
                    nc.tensor.transpose(
                        psT[:, tl:tl + 1],
                        rs4[32 * ic:32 * ic + 1, off:off + P],
                        ones_sb[32 * ic:32 * ic + 1, :],
                        tile_position=(32 * ic, 0))
                if b not in hbs:
                    hbs[b] = cpool.tile([P, TILES], f32, tag=f"h{b}",
                                        name=f"h{b}")
                hb = hbs[b]
                sl = slice(hf * TPH, (hf + 1) * TPH)
                nc.vector.tensor_add(out=hb[:, sl], in0=psT[:],
                                     in1=pre[b][:, sl])
                nc.vector.tensor_scalar_max(out=hb[:, sl], in0=hb[:, sl],
                                            scalar1=0.0)
                return hb

            # ---- phase B: per row-tile t: Eh = exp(h_t*W2) [ACT],
            # Em = (mask != 1)*Eh, accum Z [DVE], out = Em*(1/Z) [DVE],
            # store fp16. ----
            def emit_mask_load(b, g):
                m_t = mpool.tile([P, MG, N], u8, tag="mask")
                nc.sync.dma_start(m_t[:], msk[b, g])
                return m_t

            # Groups with g % 3 == 2 defer their normalize to the ACT
            # engine (emitted at the start of the next group so the ACT
            # queue never waits on the DVE) — load-balances DVE vs ACT.
            act_norm_pend = []

            def flush_act_norms():
                while act_norm_pend:
                    em, rr, o_t, u = act_norm_pend.pop(0)
                    nc.scalar.mul(o_t[:, u, :], em[:], rr[:, 0:1])

            def emit_group_compute(b, g, hb, m_t):
                flush_act_norms()
                o_t = opool.tile([P, MG, N], f16, tag="out")
                on_act = (g % 3 == 2)
                for u in range(MG):
                    t = g * MG + u
                    eh = epool.tile([P, N], f16, tag="Eh")
                    nc.scalar.activation(out=eh[:], in_=w2b_sb[:],
                                         func=AF.Exp, bias=0.0,
                                         scale=hb[:, t:t + 1])
                    em = epool.tile([P, N], f16, tag="Em")
                    zz = smpool.tile([P, 1], f32, tag="Z")
                    nc.vector.scalar_tensor_tensor(
                        out=em[:], in0=m_t[:, u, :], scalar=1.0,
                        in1=eh[:], op0=OP.not_equal, op1=OP.mult,
                        accum_out=zz[:])
                    rr = smpool.tile([P, 1], f32, tag="R")
                    nc.vector.reciprocal(rr[:], zz[:])
                    if on_act:
                        act_norm_pend.append((em, rr, o_t, u))
                    else:
                        nc.vector.tensor_scalar(out=o_t[:, u, :], in0=em[:],
                                                scalar1=rr[:, 0:1],
                                                scalar2=None, op0=OP.mult)
                return o_t

            def emit_store(b, g, o_t):
                nc.sync.dma_start(out_d[b, g], o_t[:])

            # ---- emission schedule. Single sync ring carries loads,
            # masks and stores in execution order. Batch 0's low i-half
            # loads go first so the DVE stream starts after only ~6MB;
            # batch 1's loads are front-loaded into batch 0's B-phase so
            # h1 is ready long before the DVE reaches batch 1. Stores lag
            # one group so their ring-issue never stalls earlier traffic.
            A_ORDER = [(x, jg) for x in range(3) for jg in range(NJG)]
            MPRE = 5          # mask prefetch depth (mpool bufs)

            psA0 = papool.tile([P, FC], f32, tag="psA")
            for x, jg in A_ORDER:
                emit_a_load(0, 0, x, jg, psA0)
            masks0 = {g: emit_mask_load(0, g) for g in range(MPRE)}
            h0 = emit_plumb(0, 0, psA0)
            for x, jg in A_ORDER:
                emit_a_load(0, 1, x, jg, psA0)
            emit_plumb(0, 1, psA0)

            psA1 = papool.tile([P, FC], f32, tag="psA")
            a1_iter = iter([(hf, x, jg) for hf in range(NH)
                            for x, jg in A_ORDER])
            A1_PER_G = [6, 6, 6, 6, 0, 0, 0, 0]

            masks1 = {}
            outs0 = {}
            for g in range(NMG):
                for _ in range(A1_PER_G[g]):
                    hf, x, jg = next(a1_iter)
                    emit_a_load(1, hf, x, jg, psA1)
                if g + MPRE < NMG:
                    masks0[g + MPRE] = emit_mask_load(0, g + MPRE)
                outs0[g] = emit_group_compute(0, g, h0, masks0.pop(g))
                if g == 1:
                    h1 = emit_plumb(1, 0, psA1)
                if g == 3:
                    emit_plumb(1, 1, psA1)
                    masks1.update(
                        {gg: emit_mask_load(1, gg) for gg in range(MPRE)})
                if g >= 1:
                    emit_store(0, g - 1, outs0.pop(g - 1))
            emit_store(0, NMG - 1, outs0.pop(NMG - 1))

            outs1 = {}
            for g in range(NMG):
                if g + MPRE < NMG:
                    masks1[g + MPRE] = emit_mask_load(1, g + MPRE)
                outs1[g] = emit_group_compute(1, g, h1, masks1.pop(g))
                if g >= 1:
                    emit_store(1, g - 1, outs1.pop(g - 1))
            emit_store(1, NMG - 1, outs1.pop(NMG - 1))
            flush_act_norms()

    nc.compile()
    return nc


def _ensure_ntff_hook():
    """The agent image's antenv lacks axon_hooks; inject it and register the
    boot script's ctypes NTFF hook so trace=True works."""
    import types
    if "antenv.axon_hooks" in sys.modules:
        return
    mod = types.ModuleType("antenv.axon_hooks")
    mod._hook = None

    def set_axon_ntff_profile_hook(h):
        mod._hook = h

    def get_axon_ntff_profile_hook():
        return mod._hook

    mod.set_axon_ntff_profile_hook = set_axon_ntff_profile_hook
    mod.get_axon_ntff_profile_hook = get_axon_ntff_profile_hook
    sys.modules["antenv.axon_hooks"] = mod
    try:
        from trn_agent_boot.trn_boot import _ntff_profile_via_ctypes
        mod._hook = _ntff_profile_via_ctypes('/opt/axon/libaxon_pjrt.so')
    except Exception:
        pass


def _split_pow2(k):
    """Split coefficient k into (s, alpha): k = s*alpha, s an exact
    power-of-two fp8e5 value, alpha in ~[0.707, 1.414]."""
    if k == 0.0 or not np.isfinite(k):
        return 0.0, 0.0
    e = int(np.clip(np.round(np.log2(abs(k))), STAT_EMIN, STAT_EMAX))
    s = float(np.sign(k)) * (2.0 ** e)
    return s, float(k / s)


def _quant_t(x, alpha):
    """alpha-scale, transpose [b,i,j]->[b,j,i], quantize fp8e3, and
    permute to the DMA layout [b, NH, NJG, P, JG, N2]."""
    y = np.clip(x * np.float32(alpha), -F8_CLIP, F8_CLIP)
    y = np.ascontiguousarray(y.transpose(0, 2, 1)).astype(NP_F8D)
    bsz = y.shape[0]
    return np.ascontiguousarray(
        y.reshape(bsz, NJG, JG, P, NH, N2).transpose(0, 4, 1, 3, 2, 5))


def run(inputs, trace=False):
    if trace:
        _ensure_ntff_hook()
    xe = np.asarray(inputs["expert_node"], np.float32)
    xg = np.asarray(inputs["gpu_nodes"], np.float32)
    aff = np.asarray(inputs["affinity"], np.float32)
    bwd = np.asarray(inputs["bandwidth"], np.float32)
    trf = np.asarray(inputs["traffic"], np.float32)
    msk = np.asarray(inputs["mask_gpu_action"]).astype(np.uint8)
    W_expert = np.asarray(inputs["W_expert"], np.float32)
    W_gpu = np.asarray(inputs["W_gpu"], np.float32)
    w_eatt = np.asarray(inputs["w_eatt"], np.float32)
    w_gatt = np.asarray(inputs["w_gatt"], np.float32)
    W_actor1 = np.asarray(inputs["W_actor1"], np.float32)
    W_actor2 = np.asarray(inputs["W_actor2"], np.float32)

    wa, wb, wc = w_eatt[0, 0], w_eatt[0, 1], w_eatt[0, 2]
    ga, gb = w_gatt[0, 0], w_gatt[0, 1]
    gbw, gtr = w_gatt[0, 2], w_gatt[0, 3]
    w10, w11 = W_actor1[0, 0], W_actor1[0, 1]

    consts = {
        "c_pre_e": w10 * N * wa,
        "c_pre_g": w11 * N * ga,
        "c_k0_e": w10 * wb,
        "c_k0_g": w11 * gb,
    }
    k_a = float(w10 * wc)
    k_b = float(w11 * gbw)
    k_t = float(w11 * gtr)
    s_a, al_a = _split_pow2(k_a)
    s_b, al_b = _split_pow2(k_b)
    s_t, al_t = _split_pow2(k_t)

    stat8 = np.zeros((P, 4), np.float32)
    stat8[:, 0] = s_a
    stat8[:, 1] = s_b
    stat8[:, 2] = s_t
    stat8 = stat8.astype(NP_F8S)

    a8 = _quant_t(aff, al_a)
    b8 = _quant_t(bwd, al_b)
    t8 = _quant_t(trf, al_t)
    # mask -> [B, NMG, P, MG, N]
    mskl = np.ascontiguousarray(
        msk.reshape(B, NMG, MG, P, N).transpose(0, 1, 3, 2, 4))

    u_e = W_expert[0]
    u_g = W_gpu[0]
    W2 = W_actor2[:, 0]
    w2b = np.ascontiguousarray(
        np.repeat(W2[None, :], P, 0)).astype(np.float16)
    ident = np.eye(TILES, dtype=np.float32)
    ueb = np.ascontiguousarray(
        np.broadcast_to(u_e[None, None, :], (P, TILES, DE)))
    ugb = np.ascontiguousarray(
        np.broadcast_to(u_g[None, None, :], (P, TILES, DG)))
    xe_r = np.ascontiguousarray(
        xe.reshape(B, TILES, P, DE).transpose(0, 2, 1, 3))
    xg_r = np.ascontiguousarray(
        xg.reshape(B, TILES, P, DG).transpose(0, 2, 1, 3))

    nc = _build_nc(consts)

    in_maps = []
    for c in range(NCORES):
        s = slice(c * BB, (c + 1) * BB)
        in_maps.append({
            "aff8": a8[s], "bwd8": b8[s], "trf8": t8[s],
            "mask": mskl[s], "xe": xe_r[s], "xg": xg_r[s],
            "w2b": w2b, "ueb": ueb, "ugb": ugb,
            "stat8": stat8, "ident": ident,
        })

    res = run_bass_kernel_spmd(nc, in_maps, list(range(NCORES)), trace=trace)
    # out [BB, NMG, P, MG, N] -> [B, N, N] f32
    out = np.concatenate(
        [res.results[c]["out"].transpose(0, 1, 3, 2, 4).reshape(BB, N, N)
         for c in range(NCORES)], axis=0).astype(np.float32)
    return out, res


def kernel(**inputs):
    out, _ = run(inputs, trace=False)
    return out


# revision 18
# speedup vs baseline: 1.0233x; 1.0233x over previous
"""Trainium2 Bass kernel for nn_GPU_Actor (gnn_message_passing).

Math (H=1 collapses the whole network to per-row scalars):
  Edot[b,i] = expert_node[b,i,:] . W_expert[0,:]
  Gdot[b,i] = gpu_nodes[b,i,:]  . W_gpu[0,:]
  A[b,i]  = sum_j affinity[b,i,j]   (same for bandwidth/traffic)
  h[b,i] = relu( c_pre_e*Edot + c_pre_g*Gdot + c_k0_e*Se + c_k0_g*Sg
                 + k_a*A + k_b*Bs + k_t*Ts )
  out[b,i,g] = mask[b,i,g] ? 0 : exp(h[b,i]*W2[g]) / Z[b,i]
  Z[b,i] = sum_g (1-mask) * exp(h[b,i]*W2[g])

Memory-bound; byte-count is everything:
  - affinity/bandwidth/traffic only enter via row sums -> shipped as
    fp8(e3m4), host-transposed to [j, i] layout so TensorE does the row
    sums as ones-vector matmuls (PSUM accumulation over j-chunks). The
    k_a/k_b/k_t coefficients are folded into per-tensor power-of-two fp8
    stationaries (exact) with the residual ratio folded into the fp8 data,
    so one PSUM bank accumulates k_a*A + k_b*Bs + k_t*Ts directly. The 4
    i-chunk matmuls go to distinct 32-col strips (tile_position) so they
    overlap on the array.
  - mask ships uint8, output is fp16 (host-upcast). ~50MB DMA per core.
  - All big tensors are host-permuted so every DMA descriptor is a >=4KB
    contiguous run, and loads/masks/stores ride one HWDGE ring in
    execution order (emission order = drain order, no round-robin
    dilution of the critical input stream).

Sharding: data-parallel over batch B=16 across 8 cores (2 batches/core).
"""
import sys

sys.path.insert(0, '/opt/trn_rl_repo')

import ml_dtypes
import numpy as np

import concourse.bacc as bacc
import concourse.mybir as mybir
from concourse.bass_isa import ReduceOp
from concourse.bass_utils import run_bass_kernel_spmd
from concourse.tile import TileContext

B, N, DE, DG = 16, 2048, 16, 8
NCORES = 8
BB = B // NCORES          # batches per core
P = 128                   # partitions
TILES = N // P            # 16 row-tiles per batch
JG = 4                    # j-chunks per input DMA (512 rows)
NJG = TILES // JG         # input DMAs per (batch-tensor, i-half)
NH = 2                    # i-halves: row sums finish per half
N2 = N // NH              # 1024
IC = 4                    # 512-col chunks -> 4 PE col strips
ICH = IC // NH            # col strips per half
FC = N // IC              # 512
MG = 2                    # row-tiles per output work group
NMG = TILES // MG         # 8 groups per batch
TPH = TILES // NH         # row-tiles per half

f32 = mybir.dt.float32
f16 = mybir.dt.float16
f8d = mybir.dt.float8e3   # e3m4: data dtype (4 mantissa bits)
f8s = mybir.dt.float8e5   # e5m2: stationary dtype (pow2 exact)
u8 = mybir.dt.uint8
AX = mybir.AxisListType
OP = mybir.AluOpType
AF = mybir.ActivationFunctionType

NP_F8D = ml_dtypes.float8_e3m4
NP_F8S = ml_dtypes.float8_e5m2
F8_CLIP = 15.0            # e3m4 max normal is 15.5
STAT_EMIN, STAT_EMAX = -14, 15


def _build_nc(consts):
    c_pre_e = float(consts["c_pre_e"])
    c_pre_g = float(consts["c_pre_g"])
    c_k0_e = float(consts["c_k0_e"])
    c_k0_g = float(consts["c_k0_g"])

    nc = bacc.Bacc("TRN2", target_bir_lowering=False, debug=False,
                   num_devices=NCORES)

    # inputs, host-permuted:
    #   big fp8 [BB, NH, NJG, P, JG, N2]: row (jg*JG*P + u*P + p), col
    #     (h*N2 + n) of the transposed [j, i] tensor at [b, h, jg, p, u, n]
    #     -- i-halves stored separately so the low half's row sums (and
    #     thus h[0:1024]) complete after only half the input bytes.
    #   mask [BB, NMG, P, MG, N]: row (g*MG*P + u*P + p) at [b, g, p, u, :]
    #   out  [BB, NMG, P, MG, N] fp16, same permutation (host undoes it)
    a8 = nc.dram_tensor("aff8", [BB, NH, NJG, P, JG, N2], f8d,
                        kind="ExternalInput")
    b8 = nc.dram_tensor("bwd8", [BB, NH, NJG, P, JG, N2], f8d,
                        kind="ExternalInput")
    t8 = nc.dram_tensor("trf8", [BB, NH, NJG, P, JG, N2], f8d,
                        kind="ExternalInput")
    msk = nc.dram_tensor("mask", [BB, NMG, P, MG, N], u8,
                         kind="ExternalInput")
    xe = nc.dram_tensor("xe", [BB, P, TILES, DE], f32, kind="ExternalInput")
    xg = nc.dram_tensor("xg", [BB, P, TILES, DG], f32, kind="ExternalInput")
    w2b = nc.dram_tensor("w2b", [P, N], f16, kind="ExternalInput")
    ueb = nc.dram_tensor("ueb", [P, TILES, DE], f32, kind="ExternalInput")
    ugb = nc.dram_tensor("ugb", [P, TILES, DG], f32, kind="ExternalInput")
    st8 = nc.dram_tensor("stat8", [P, 4], f8s, kind="ExternalInput")
    out_d = nc.dram_tensor("out", [BB, NMG, P, MG, N], f16,
                           kind="ExternalOutput")

    with TileContext(nc) as tc:
        with tc.tile_pool(name="const", bufs=1) as cpool, \
             tc.tile_pool(name="stream", bufs=3) as spool, \
             tc.tile_pool(name="mpool", bufs=12) as mpool, \
             tc.tile_pool(name="epool", bufs=4) as epool, \
             tc.tile_pool(name="opool", bufs=3) as opool, \
             tc.tile_pool(name="small", bufs=6) as smpool, \
             tc.tile_pool(name="psA", bufs=1, space="PSUM") as papool, \
             tc.tile_pool(name="psT", bufs=2, space="PSUM") as ptpool:

            w2b_sb = cpool.tile([P, N], f16, tag="w2b")
            nc.scalar.dma_start(w2b_sb[:], w2b[:])
            st_sb = cpool.tile([P, 4], f8s, tag="stat8")
            nc.scalar.dma_start(st_sb[:], st8[:])
            ones_sb = cpool.tile([P, 1], f32, tag="ones")
            nc.vector.memset(ones_sb[:], 1.0)
            ue_sb = cpool.tile([P, TILES, DE], f32, tag="ueb")
            nc.scalar.dma_start(ue_sb[:], ueb[:])
            ug_sb = cpool.tile([P, TILES, DG], f32, tag="ugb")
            nc.scalar.dma_start(ug_sb[:], ugb[:])

            # warm the ACT exp table before it's on the critical path
            warm = smpool.tile([P, 1], f32, tag="warm")
            nc.scalar.activation(out=warm[:], in_=ue_sb[:, 0, 0:1],
                                 func=AF.Exp, bias=0.0, scale=0.0)

            # ---- stage 1: per-batch row scalars from tiny xe/xg ----
            pre = []
            for b in range(BB):
                xe_sb = cpool.tile([P, TILES, DE], f32, tag=f"xe{b}")
                nc.scalar.dma_start(xe_sb[:], xe[b])
                xg_sb = cpool.tile([P, TILES, DG], f32, tag=f"xg{b}")
                nc.scalar.dma_start(xg_sb[:], xg[b])

                prod_e = smpool.tile([P, TILES, DE], f32, tag="prod_e")
                nc.vector.tensor_mul(out=prod_e[:], in0=xe_sb[:], in1=ue_sb[:])
                edot = cpool.tile([P, TILES], f32, tag=f"edot{b}")
                nc.vector.tensor_reduce(out=edot[:], in_=prod_e[:],
                                        axis=AX.X, op=OP.add)
                prod_g = smpool.tile([P, TILES, DG], f32, tag="prod_g")
                nc.vector.tensor_mul(out=prod_g[:], in0=xg_sb[:], in1=ug_sb[:])
                gdot = cpool.tile([P, TILES], f32, tag=f"gdot{b}")
                nc.vector.tensor_reduce(out=gdot[:], in_=prod_g[:],
                                        axis=AX.X, op=OP.add)

                sep = smpool.tile([P, 1], f32, tag="sep")
                nc.vector.tensor_reduce(out=sep[:], in_=edot[:],
                                        axis=AX.X, op=OP.add)
                sgp = smpool.tile([P, 1], f32, tag="sgp")
                nc.vector.tensor_reduce(out=sgp[:], in_=gdot[:],
                                        axis=AX.X, op=OP.add)
                sea = smpool.tile([P, 1], f32, tag="sea")
                nc.gpsimd.partition_all_reduce(sea[:], sep[:], channels=P,
                                               reduce_op=ReduceOp.add)
                sga = smpool.tile([P, 1], f32, tag="sga")
                nc.gpsimd.partition_all_reduce(sga[:], sgp[:], channels=P,
                                               reduce_op=ReduceOp.add)

                k0 = smpool.tile([P, 1], f32, tag="k0")
                nc.vector.tensor_scalar(out=k0[:], in0=sea[:],
                                        scalar1=c_k0_e, scalar2=None,
                                        op0=OP.mult)
                k0b = cpool.tile([P, 1], f32, tag=f"k0b{b}")
                nc.vector.tensor_scalar(out=k0b[:], in0=sga[:],
                                        scalar1=c_k0_g, scalar2=k0[:, 0:1],
                                        op0=OP.mult, op1=OP.add)
                pre_b = cpool.tile([P, TILES], f32, tag=f"pre{b}")
                nc.vector.tensor_scalar(out=pre_b[:], in0=edot[:],
                                        scalar1=c_pre_e, scalar2=k0b[:, 0:1],
                                        op0=OP.mult, op1=OP.add)
                nc.vector.scalar_tensor_tensor(out=pre_b[:], in0=gdot[:],
                                               scalar=c_pre_g, in1=pre_b[:],
                                               op0=OP.mult, op1=OP.add)
                pre.append(pre_b)

            # ---- phase A: TensorE row sums. One input DMA = [P, JG, N2]
            # fp8 (4KB contiguous per partition); per (x, jg, u) the two
            # strip matmuls of this half go to distinct 32-col strips of
            # the array and accumulate into psA[32*ic, :]. ----
            SRCS = (a8, b8, t8)

            def emit_a_load(b, hf, x, jg, psA):
                # psA is this (batch, half)'s own PSUM tile (own bank) so
                # the lo-half plumb does not wait on hi-half matmuls.
                d_t = spool.tile([P, JG, N2], f8d, tag=f"in{x}")
                nc.sync.dma_start(d_t[:], SRCS[x][b, hf, jg])
                for u in range(JG):
                    for icl in range(ICH):
                        nc.tensor.matmul(
                            psA[32 * icl:32 * icl + 1, :],
                            lhsT=st_sb[:, x:x + 1],
                            rhs=d_t[:, u, icl * FC:(icl + 1) * FC],
                            start=(x == 0 and jg == 0 and u == 0),
                            stop=(x == 2 and jg == NJG - 1 and u == JG - 1),
                            tile_position=(0, 32 * icl))

            hbs = {}

            def emit_plumb(b, hf, psA):
                # PSUM strips of this half -> SBUF (partition-aligned ACT
                # copies), then tiny PE transposes [1,128] -> [128,1]
                # spread the per-row sums across partitions; add pre, relu
                # on the (otherwise idle) GPSIMD so the DVE FIFO is never
                # blocked waiting for phase-A data.
                rs4 = smpool.tile([P, FC], f32, tag=f"rs4_{hf}")
                for icl in range(ICH):
                    nc.scalar.copy(rs4[32 * icl:32 * icl + 1, :],
                                   psA[32 * icl:32 * icl + 1, :])
                psT = ptpool.tile([P, TPH], f32, tag="psT")
                for tl in range(TPH):
                    icl = tl // IC
                    off = (tl % IC) * P
                    nc.tensor.transpose(
                        psT[:, tl:tl + 1],
                        rs4[32 * icl:32 * icl + 1, off:off + P],
                        ones_sb[32 * icl:32 * icl + 1, :],
                        tile_position=(32 * icl, 0))
                psT_sb = smpool.tile([P, TPH], f32, tag=f"psTsb{hf}")
                nc.scalar.copy(psT_sb[:], psT[:])
                if b not in hbs:
                    hbs[b] = cpool.tile([P, TILES], f32, tag=f"h{b}",
                                        name=f"h{b}")
                hb = hbs[b]
                sl = slice(hf * TPH, (hf + 1) * TPH)
                nc.gpsimd.tensor_add(out=hb[:, sl], in0=psT_sb[:],
                                     in1=pre[b][:, sl])
                nc.gpsimd.tensor_scalar_max(out=hb[:, sl], in0=hb[:, sl],
                                            scalar1=0.0)
                return hb

            # ---- phase B: per row-tile t: Eh = exp(h_t*W2) [ACT],
            # Em = (mask != 1)*Eh, accum Z [DVE], out = Em*(1/Z) [DVE],
            # store fp16. ----
            def emit_mask_load(b, g):
                m_t = mpool.tile([P, MG, N], u8, tag="mask")
                nc.sync.dma_start(m_t[:], msk[b, g])
                return m_t

            # Groups with g % 3 == 2 defer their normalize to the ACT
            # engine (emitted at the start of the next group so the ACT
            # queue never waits on the DVE) — load-balances DVE vs ACT.
            act_norm_pend = []

            def flush_act_norms():
                while act_norm_pend:
                    em, rr, o_t, u = act_norm_pend.pop(0)
                    nc.scalar.mul(o_t[:, u, :], em[:], rr[:, 0:1])

            def emit_group_compute(b, g, hb, m_t):
                flush_act_norms()
                o_t = opool.tile([P, MG, N], f16, tag="out")
                on_act = (g % 3 == 2)
                for u in range(MG):
                    t = g * MG + u
                    eh = epool.tile([P, N], f16, tag="Eh")
                    nc.scalar.activation(out=eh[:], in_=w2b_sb[:],
                                         func=AF.Exp, bias=0.0,
                                         scale=hb[:, t:t + 1])
                    em = epool.tile([P, N], f16, tag="Em")
                    zz = smpool.tile([P, 1], f32, tag="Z")
                    nc.vector.scalar_tensor_tensor(
                        out=em[:], in0=m_t[:, u, :], scalar=1.0,
                        in1=eh[:], op0=OP.not_equal, op1=OP.mult,
                        accum_out=zz[:])
                    rr = smpool.tile([P, 1], f32, tag="R")
                    nc.vector.reciprocal(rr[:], zz[:])
                    if on_act:
                        act_norm_pend.append((em, rr, o_t, u))
                    else:
                        nc.vector.tensor_scalar(out=o_t[:, u, :], in0=em[:],
                                                scalar1=rr[:, 0:1],
                                                scalar2=None, op0=OP.mult)
                return o_t

            def emit_store(b, g, o_t):
                nc.sync.dma_start(out_d[b, g], o_t[:])

            # ---- emission schedule. Single sync ring carries loads,
            # masks and stores in execution order. Batch 0's low i-half
            # loads go first so the DVE stream starts after only ~6MB;
            # batch 1's loads are front-loaded into batch 0's B-phase so
            # h1 is ready long before the DVE reaches batch 1. Stores lag
            # one group so their ring-issue never stalls earlier traffic.
            A_ORDER = [(x, jg) for x in range(3) for jg in range(NJG)]

            # batch 0: lo-half loads, all 8 masks, hi-half loads.
            psA00 = papool.tile([P, FC], f32, tag="psA0")
            for x, jg in A_ORDER:
                emit_a_load(0, 0, x, jg, psA00)
            masks0 = {g: emit_mask_load(0, g) for g in range(NMG)}
            h0 = emit_plumb(0, 0, psA00)
            psA01 = papool.tile([P, FC], f32, tag="psA1")
            for x, jg in A_ORDER:
                emit_a_load(0, 1, x, jg, psA01)
            emit_plumb(0, 1, psA01)

            # batch 1 loads front-loaded into batch 0's B-phase; masks for
            # batch 1 interleaved so they land just before the DVE needs
            # them without displacing the h1-critical loads.
            psA10 = papool.tile([P, FC], f32, tag="psA0")
            psA11 = papool.tile([P, FC], f32, tag="psA1")
            a1_iter = iter([(hf, x, jg) for hf in range(NH)
                            for x, jg in A_ORDER])
            A1_PER_G = [4, 4, 4, 4, 4, 4, 0, 0]
            M1_AT_G = {2: [0, 1], 3: [2, 3, 4], 4: [5, 6, 7]}

            masks1 = {}
            outs0 = {}
            for g in range(NMG):
                for gg in M1_AT_G.get(g, []):
                    masks1[gg] = emit_mask_load(1, gg)
                for _ in range(A1_PER_G[g]):
                    hf, x, jg = next(a1_iter)
                    emit_a_load(1, hf, x, jg, psA10 if hf == 0 else psA11)
                outs0[g] = emit_group_compute(0, g, h0, masks0.pop(g))
                if g == 2:
                    h1 = emit_plumb(1, 0, psA10)
                if g == 5:
                    emit_plumb(1, 1, psA11)
                if g >= 1:
                    emit_store(0, g - 1, outs0.pop(g - 1))
            emit_store(0, NMG - 1, outs0.pop(NMG - 1))

            outs1 = {}
            for g in range(NMG):
                outs1[g] = emit_group_compute(1, g, h1, masks1.pop(g))
                if g >= 1:
                    emit_store(1, g - 1, outs1.pop(g - 1))
            emit_store(1, NMG - 1, outs1.pop(NMG - 1))
            flush_act_norms()

    nc.compile()
    return nc


def _ensure_ntff_hook():
    """The agent image's antenv lacks axon_hooks; inject it and register the
    boot script's ctypes NTFF hook so trace=True works."""
    import types
    if "antenv.axon_hooks" in sys.modules:
        return
    mod = types.ModuleType("antenv.axon_hooks")
    mod._hook = None

    def set_axon_ntff_profile_hook(h):
        mod._hook = h

    def get_axon_ntff_profile_hook():
        return mod._hook

    mod.set_axon_ntff_profile_hook = set_axon_ntff_profile_hook
    mod.get_axon_ntff_profile_hook = get_axon_ntff_profile_hook
    sys.modules["antenv.axon_hooks"] = mod
    try:
        from trn_agent_boot.trn_boot import _ntff_profile_via_ctypes
        mod._hook = _ntff_profile_via_ctypes('/opt/axon/libaxon_pjrt.so')
    except Exception:
        pass


def _split_pow2(k):
    """Split coefficient k into (s, alpha): k = s*alpha, s an exact
    power-of-two fp8e5 value, alpha in ~[0.707, 1.414]."""
    if k == 0.0 or not np.isfinite(k):
        return 0.0, 0.0
    e = int(np.clip(np.round(np.log2(abs(k))), STAT_EMIN, STAT_EMAX))
    s = float(np.sign(k)) * (2.0 ** e)
    return s, float(k / s)


def _quant_t(x, alpha):
    """alpha-scale, transpose [b,i,j]->[b,j,i], quantize fp8e3, and
    permute to the DMA layout [b, NH, NJG, P, JG, N2]."""
    y = np.clip(x * np.float32(alpha), -F8_CLIP, F8_CLIP)
    y = np.ascontiguousarray(y.transpose(0, 2, 1)).astype(NP_F8D)
    bsz = y.shape[0]
    return np.ascontiguousarray(
        y.reshape(bsz, NJG, JG, P, NH, N2).transpose(0, 4, 1, 3, 2, 5))


def run(inputs, trace=False):
    if trace:
        _ensure_ntff_hook()
    xe = np.asarray(inputs["expert_node"], np.float32)
    xg = np.asarray(inputs["gpu_nodes"], np.float32)
    aff = np.asarray(inputs["affinity"], np.float32)
    bwd = np.asarray(inputs["bandwidth"], np.float32)
    trf = np.asarray(inputs["traffic"], np.float32)
    msk = np.asarray(inputs["mask_gpu_action"]).astype(np.uint8)
    W_expert = np.asarray(inputs["W_expert"], np.float32)
    W_gpu = np.asarray(inputs["W_gpu"], np.float32)
    w_eatt = np.asarray(inputs["w_eatt"], np.float32)
    w_gatt = np.asarray(inputs["w_gatt"], np.float32)
    W_actor1 = np.asarray(inputs["W_actor1"], np.float32)
    W_actor2 = np.asarray(inputs["W_actor2"], np.float32)

    wa, wb, wc = w_eatt[0, 0], w_eatt[0, 1], w_eatt[0, 2]
    ga, gb = w_gatt[0, 0], w_gatt[0, 1]
    gbw, gtr = w_gatt[0, 2], w_gatt[0, 3]
    w10, w11 = W_actor1[0, 0], W_actor1[0, 1]

    consts = {
        "c_pre_e": w10 * N * wa,
        "c_pre_g": w11 * N * ga,
        "c_k0_e": w10 * wb,
        "c_k0_g": w11 * gb,
    }
    k_a = float(w10 * wc)
    k_b = float(w11 * gbw)
    k_t = float(w11 * gtr)
    s_a, al_a = _split_pow2(k_a)
    s_b, al_b = _split_pow2(k_b)
    s_t, al_t = _split_pow2(k_t)

    stat8 = np.zeros((P, 4), np.float32)
    stat8[:, 0] = s_a
    stat8[:, 1] = s_b
    stat8[:, 2] = s_t
    stat8 = stat8.astype(NP_F8S)

    a8 = _quant_t(aff, al_a)
    b8 = _quant_t(bwd, al_b)
    t8 = _quant_t(trf, al_t)
    # mask -> [B, NMG, P, MG, N]
    mskl = np.ascontiguousarray(
        msk.reshape(B, NMG, MG, P, N).transpose(0, 1, 3, 2, 4))

    u_e = W_expert[0]
    u_g = W_gpu[0]
    W2 = W_actor2[:, 0]
    w2b = np.ascontiguousarray(
        np.repeat(W2[None, :], P, 0)).astype(np.float16)
    ident = np.eye(TILES, dtype=np.float32)
    ueb = np.ascontiguousarray(
        np.broadcast_to(u_e[None, None, :], (P, TILES, DE)))
    ugb = np.ascontiguousarray(
        np.broadcast_to(u_g[None, None, :], (P, TILES, DG)))
    xe_r = np.ascontiguousarray(
        xe.reshape(B, TILES, P, DE).transpose(0, 2, 1, 3))
    xg_r = np.ascontiguousarray(
        xg.reshape(B, TILES, P, DG).transpose(0, 2, 1, 3))

    nc = _build_nc(consts)

    in_maps = []
    for c in range(NCORES):
        s = slice(c * BB, (c + 1) * BB)
        in_maps.append({
            "aff8": a8[s], "bwd8": b8[s], "trf8": t8[s],
            "mask": mskl[s], "xe": xe_r[s], "xg": xg_r[s],
            "w2b": w2b, "ueb": ueb, "ugb": ugb,
            "stat8": stat8, "ident": ident,
        })

    res = run_bass_kernel_spmd(nc, in_maps, list(range(NCORES)), trace=trace)
    # out [BB, NMG, P, MG, N] -> [B, N, N] f32
    out = np.concatenate(
        [res.results[c]["out"].transpose(0, 1, 3, 2, 4).reshape(BB, N, N)
         for c in range(NCORES)], axis=0).astype(np.float32)
    return out, res


def kernel(**inputs):
    out, _ = run(inputs, trace=False)
    return out
